# revision 12
# baseline (speedup 1.0000x reference)
"""Trainium2 Bass kernel: fused concat-linear attention map + softmax.

reference:  scores[b,h,n] = key[b,n,:]@Wk[h,:] + query[b,0,:]@Wq[h,:] + bias[h]
            attn = softmax over n              (B=16, N=20000, D=256, H=8)

Sharding: batch dim B=16 split across 8 cores (2 batches/core), weights
replicated.  Per batch the kernel streams key (20.5 MB f32) through:

  DMA (2 MB chunks, natural layout)
    -> PE transpose of 128x128 blocks in f32r (1.5 cyc/row vs 2.0 for f32)
    -> DVE/ACT copy PSUM->SBUF (one half-chunk each, balancing the load)
    -> PE matmul vs slot-placed stationaries: for chunk slot m (= chunk
       index mod 16) the stationary is a [128,128] matrix whose columns
       8m..8m+8 hold WkT-half and all other columns are zero.  Each matmul
       therefore writes its 8 score rows to PSUM partitions 8m..8m+8 while
       accumulating exact zeros everywhere else, packing 16 chunks of
       scores into one [128,512] PSUM bank.
    -> ONE ScalarE exp per 16 chunks on all 128 lanes, fused bias
       (qWq+b replicated per-partition) and accumulated row sums.
       (softmax without max-subtraction: scores are O(+-7) so f32 exp is
       safe and mathematically identical)
    -> per-h totals via a tiny pattern matmul (sums live on 128
       partitions), reciprocal, replicate back to 128 partitions,
       one 128-lane scale, strided DMA out.

The PE stream is software-pipelined: transposes for chunk c+PIPE are
issued before the matmuls of chunk c, so PE never stalls waiting for the
PSUM->SBUF copy of the chunk it is about to multiply.  Batch 0's scale +
writeback run on the GPSIMD queue (SWDGE DMA) so SP keeps issuing batch
1's loads without a bubble.
"""

import sys

import numpy as np

for _p in ("/opt/trn_rl_repo",):
    if _p not in sys.path:
        sys.path.append(_p)

from contextlib import ExitStack

import concourse.bass as bass
import concourse.bacc as bacc
import concourse.tile as tile
from concourse import mybir
from concourse.masks import make_identity

B, N, D, H = 16, 20000, 256, 8
NCORES = 8
BPC = B // NCORES  # batches per core
P = 128
CHUNK = 512  # n-columns per score chunk (= one PSUM bank of f32)
GROUP = P // H  # chunks whose scores pack into one PSUM bank (16)
LOAD_SUB = 16  # 128-row subtiles per load DMA (2048 rows = 2 MB)
PIPE = 2  # chunks of PE transpose lookahead ahead of the score matmuls
F32 = mybir.dt.float32
F32R = mybir.dt.float32r
NEG_BIG = -1e30  # exp(NEG_BIG) == 0: masks ragged-tail pad columns


def _ceil_div(a, b):
    return (a + b - 1) // b


def build_kernel(n=N, bpc=BPC):
    nc = bacc.Bacc("TRN2", target_bir_lowering=False, debug=False)
    q_in = nc.declare_dram_parameter("q", [bpc, D], F32, isOutput=False)
    k_in = nc.declare_dram_parameter("k", [bpc, n, D], F32, isOutput=False)
    w_in = nc.declare_dram_parameter("w", [H, 2 * D], F32, isOutput=False)
    b_in = nc.declare_dram_parameter("b", [H], F32, isOutput=False)
    out = nc.declare_dram_parameter("out", [bpc, H, n], F32, isOutput=True)

    nchunks = _ceil_div(n, CHUNK)
    ngroups = _ceil_div(nchunks, GROUP)
    nfullg = nchunks // GROUP  # groups with all 16 slots populated
    last_slots = nchunks - (ngroups - 1) * GROUP
    w_last = n - (nchunks - 1) * CHUNK  # width of the ragged tail chunk
    load_rows = LOAD_SUB * P

    def _r(ap):  # bitcast f32 -> f32r (same bits, faster PE moving rate)
        return ap.bitcast(F32R)

    with ExitStack() as ctx:
        tc = ctx.enter_context(tile.TileContext(nc))
        consts = ctx.enter_context(tc.tile_pool(name="consts", bufs=1))
        loads = ctx.enter_context(tc.tile_pool(name="loads", bufs=3))
        kts = ctx.enter_context(tc.tile_pool(name="kts", bufs=PIPE + 1))
        probp = ctx.enter_context(tc.tile_pool(name="prob", bufs=2))
        small = ctx.enter_context(tc.tile_pool(name="small", bufs=2))
        # PSUM budget (8 banks): kt0,kt1 x3 bufs = 6 banks + sc x2 = 2 banks.
        psum = ctx.enter_context(tc.tile_pool(name="psum", bufs=1, space="PSUM"))

        def sc_tile(shape, name):
            return psum.tile(shape, F32, tag="sc", bufs=2, name=name)

        identity = consts.tile([P, P], F32)
        make_identity(nc, identity)
        idr = consts.tile([P, P], F32R)
        nc.vector.tensor_copy(out=idr[:, :], in_=identity[:, :])

        # --- constants: weights, bias, queries -----------------------------
        w_sb = consts.tile([H, 2 * D], F32)
        nc.sync.dma_start(out=w_sb[:, :], in_=w_in[:, :])
        b_sb = consts.tile([H, 1], F32)
        nc.sync.dma_start(out=b_sb[:, :], in_=b_in[:])

        # wqT[:, c, :] / wkT[:, c, :]: the [d,h] transposed halves of W.
        wqT = consts.tile([P, 2, H], F32)
        wkT = consts.tile([P, 2, H], F32)
        for c in range(4):
            pt = sc_tile([P, H], "pt")
            nc.tensor.transpose(pt[:, :], w_sb[:, c * P:(c + 1) * P], identity[:H, :H])
            dst = wqT[:, c, :] if c < 2 else wkT[:, c - 2, :]
            nc.vector.tensor_copy(out=dst, in_=pt[:, :])

        # Slot-placed stationaries: wkS[:, c, m, 8m:8m+8] = wkT[:, c, :],
        # zero elsewhere.  matmul vs wkS[:, c, m, :] lands scores on PSUM
        # partitions 8m..8m+8 and accumulates zeros on the rest.
        wkS = consts.tile([P, 2, GROUP, P], F32R)
        # memset can't target f32r: zero a scratch f32 tile (borrowed from
        # the loads pool, recycled afterwards) and cast-copy it in.
        zscratch = loads.tile([P, LOAD_SUB, D], F32, tag="load", name="zscratch")
        nc.vector.memset(zscratch[:, :, :], 0.0)
        nc.vector.tensor_copy(
            out=wkS[:, :, :, :].rearrange("p a b c -> p (a b c)"),
            in_=zscratch[:, :, :].rearrange("p s d -> p (s d)"),
        )
        for c in range(2):
            for m in range(GROUP):
                nc.vector.tensor_copy(
                    out=wkS[:, c, m, H * m:H * m + H], in_=wkT[:, c, :]
                )

        # patT[p, h] = 1 if p % H == h  (for summing slot rows back to h);
        # repT = patT^T (for replicating [H,1] vectors to 128 partitions).
        patT = consts.tile([P, H], F32)
        nc.vector.memset(patT[:, :], 0.0)
        for m in range(GROUP):
            nc.vector.tensor_add(
                patT[:, :], patT[:, :], identity[:, H * m:H * m + H]
            )
        repT = consts.tile([H, P], F32)
        rp = sc_tile([H, P], "rp")
        nc.tensor.transpose(rp[:, :], patT[:, :], identity[:, :])
        nc.vector.tensor_copy(out=repT[:, :], in_=rp[:, :])

        q_sb = consts.tile([1, bpc, D], F32)
        nc.sync.dma_start(out=q_sb[:, :, :], in_=q_in[:, :])
        qT = consts.tile([P, bpc, 2], F32)
        for i in range(bpc):
            for c in range(2):
                pt = sc_tile([P, 1], "pt2")
                nc.tensor.transpose(
                    pt[:, :], q_sb[0:1, i, c * P:(c + 1) * P], identity[:1, :1]
                )
                nc.vector.tensor_copy(out=qT[:, i, c:c + 1], in_=pt[:, :])

        # qb[:, i] = Wq @ q_i + b; qb2 = qb replicated to all 128 partitions.
        qb = consts.tile([H, bpc], F32)
        for i in range(bpc):
            qp = sc_tile([H, 1], "qp")
            nc.tensor.matmul(
                qp[:, :], wqT[:, 0, :], qT[:, i, 0:1], start=True, stop=False
            )
            nc.tensor.matmul(
                qp[:, :], wqT[:, 1, :], qT[:, i, 1:2], start=False, stop=True
            )
            nc.vector.tensor_add(qb[:, i:i + 1], qp[:, :], b_sb[:, :])
        qb2 = consts.tile([P, bpc], F32)
        qb2p = sc_tile([P, bpc], "qb2p")
        nc.tensor.matmul(qb2p[:, :], repT[:, :], qb[:, :], start=True, stop=True)
        nc.vector.tensor_copy(out=qb2[:, :], in_=qb2p[:, :])

        # --- main loop: flat chunk pipeline per batch ----------------------
        for i in range(bpc):
            prob2 = probp.tile([P, ngroups * CHUNK], F32, tag="prob")
            nsumcol = ngroups + (1 if w_last < CHUNK else 0)
            sums2 = small.tile([P, nsumcol], F32, tag="sums")
            if last_slots < GROUP:
                # partitions no slot of the last group writes: zero their sums
                nc.vector.memset(sums2[H * last_slots:, ngroups - 1:], 0.0)
            if w_last < CHUNK:
                # the split tail exp only writes partitions [0, H*(last_slots-1))
                nc.vector.memset(sums2[:H * last_slots, nsumcol - 1:nsumcol], 0.0)
            ld_tiles = {}
            pend = []
            scps = {}
            for step in range(nchunks + PIPE):
                if step < nchunks:
                    n0 = step * CHUNK
                    L = n0 // load_rows
                    if n0 % load_rows == 0:
                        rows = min(load_rows, n - L * load_rows)
                        full_sub = rows // P
                        rem = rows - full_sub * P
                        ld = loads.tile([P, LOAD_SUB, D], F32R, tag="load")
                        if full_sub:
                            nc.sync.dma_start(
                                out=ld[:, :full_sub, :],
                                in_=k_in[
                                    i, L * load_rows:L * load_rows + full_sub * P, :
                                ].rearrange("(s p) d -> p s d", p=P).bitcast(F32R),
                            )
                        if rem:
                            nc.sync.dma_start(
                                out=ld[:rem, full_sub, :],
                                in_=k_in[
                                    i,
                                    L * load_rows + full_sub * P:L * load_rows + rows,
                                    :,
                                ].bitcast(F32R),
                            )
                        ld_tiles[L] = ld
                    ld = ld_tiles[L]
                    w = min(CHUNK, n - n0)
                    kt0 = psum.tile([P, CHUNK], F32R, tag="kt0", bufs=3, name="kt0")
                    kt1 = psum.tile([P, CHUNK], F32R, tag="kt1", bufs=3, name="kt1")
                    for t in range(_ceil_div(w, P)):
                        tw = min(P, w - t * P)
                        s = (n0 % load_rows) // P + t
                        nc.tensor.transpose(
                            kt0[:, t * P:t * P + tw],
                            ld[:tw, s, 0:P],
                            idr[:tw, :tw],
                        )
                        nc.tensor.transpose(
                            kt1[:, t * P:t * P + tw],
                            ld[:tw, s, P:2 * P],
                            idr[:tw, :tw],
                        )
                    k0 = kts.tile([P, CHUNK], F32R, tag="k0", name="k0")
                    k1 = kts.tile([P, CHUNK], F32R, tag="k1", name="k1")
                    nc.vector.tensor_copy(out=k0[:, :w], in_=kt0[:, :w])
                    nc.scalar.copy(out=k1[:, :w], in_=kt1[:, :w])
                    pend.append((k0, k1, w))
                if step >= PIPE:
                    c = step - PIPE
                    gg, m = c // GROUP, c % GROUP
                    if m == 0:
                        scps[gg] = sc_tile([P, CHUNK], "scp")
                    scp = scps[gg]
                    k0, k1, w = pend[c]
                    last_in_group = c == nchunks - 1 or m == GROUP - 1
                    nc.tensor.matmul(
                        scp[:, :w], wkS[:, 0, m, :], k0[:, :w],
                        start=(m == 0), stop=False,
                    )
                    nc.tensor.matmul(
                        scp[:, :w], wkS[:, 1, m, :], k1[:, :w],
                        start=False, stop=last_in_group,
                    )
                    if last_in_group:
                        nslots = m + 1
                        if w == CHUNK:
                            nc.scalar.activation(
                                out=prob2[:H * nslots, gg * CHUNK:(gg + 1) * CHUNK],
                                in_=scp[:H * nslots, :],
                                func=mybir.ActivationFunctionType.Exp,
                                bias=qb2[:H * nslots, i:i + 1],
                                scale=1.0,
                                accum_out=sums2[:H * nslots, gg:gg + 1],
                            )
                        else:
                            # ragged tail chunk: exp only the valid regions
                            # (engine partition bases must stay 32-aligned, so
                            # regions start at partition 0)
                            g0 = gg * CHUNK
                            nc.scalar.activation(
                                out=prob2[:H * nslots, g0:g0 + w],
                                in_=scp[:H * nslots, :w],
                                func=mybir.ActivationFunctionType.Exp,
                                bias=qb2[:H * nslots, i:i + 1],
                                scale=1.0,
                                accum_out=sums2[:H * nslots, gg:gg + 1],
                            )
                            nc.scalar.activation(
                                out=prob2[:H * (nslots - 1), g0 + w:g0 + CHUNK],
                                in_=scp[:H * (nslots - 1), w:],
                                func=mybir.ActivationFunctionType.Exp,
                                bias=qb2[:H * (nslots - 1), i:i + 1],
                                scale=1.0,
                                accum_out=sums2[:H * (nslots - 1), nsumcol - 1:nsumcol],
                            )

            # --- per-batch tail: totals, reciprocal, scale, writeback ------
            totp = sc_tile([H, nsumcol], "totp")
            nc.tensor.matmul(totp[:, :], patT[:, :], sums2[:, :], start=True, stop=True)
            tot = small.tile([H, 1], F32, tag="tot")
            nc.vector.reduce_sum(out=tot[:, :], in_=totp[:, :], axis=mybir.AxisListType.X)
            rec = small.tile([H, 1], F32, tag="rec")
            nc.vector.reciprocal(out=rec[:, :], in_=tot[:, :])
            rec2p = sc_tile([P, 1], "rec2p")
            nc.tensor.matmul(rec2p[:, :], repT[:, :], rec[:, :], start=True, stop=True)
            rec2 = small.tile([P, 1], F32, tag="rec2")
            nc.vector.tensor_copy(out=rec2[:, :], in_=rec2p[:, :])

            # Batch 0: scale + writeback on the GPSIMD queue (SWDGE) so SP
            # keeps issuing batch 1's loads.  Last batch: DVE + SP.
            last_batch = i == bpc - 1
            veng = nc.vector if last_batch else nc.gpsimd
            deng = nc.sync if last_batch else nc.gpsimd
            veng.tensor_scalar_mul(
                prob2[:, 0:nfullg * CHUNK],
                prob2[:, 0:nfullg * CHUNK],
                rec2[:, :],
            )
            if last_slots < GROUP:
                lg0 = nfullg * CHUNK
                if w_last < CHUNK:
                    veng.tensor_scalar_mul(
                        prob2[:H * last_slots, lg0:lg0 + w_last],
                        prob2[:H * last_slots, lg0:lg0 + w_last],
                        rec2[:H * last_slots, :],
                    )
                    veng.tensor_scalar_mul(
                        prob2[:H * (last_slots - 1), lg0 + w_last:lg0 + CHUNK],
                        prob2[:H * (last_slots - 1), lg0 + w_last:lg0 + CHUNK],
                        rec2[:H * (last_slots - 1), :],
                    )
                else:
                    veng.tensor_scalar_mul(
                        prob2[:H * last_slots, lg0:lg0 + CHUNK],
                        prob2[:H * last_slots, lg0:lg0 + CHUNK],
                        rec2[:H * last_slots, :],
                    )

            # Writeback: the SBUF side of a DMA cannot split the partition
            # dim, so issue one DMA per slot (plain [8, ...] partition slice;
            # the DRAM side carries the group stride).  Round-robin the issue
            # queues so the tail isn't serialized on one sequencer.
            span0 = nfullg * GROUP * CHUNK
            dengs = [nc.gpsimd] if not last_batch else [nc.sync, nc.scalar, nc.gpsimd]
            out_full = out[i, :, 0:span0].rearrange(
                "h (g m j) -> m h g j", g=nfullg, m=GROUP, j=CHUNK
            )
            di = 0
            for m in range(GROUP):
                dengs[di % len(dengs)].dma_start(
                    out=out_full[m],
                    in_=prob2[H * m:H * (m + 1), 0:nfullg * CHUNK].rearrange(
                        "h (g j) -> h g j", j=CHUNK
                    ),
                )
                di += 1
            if last_slots:
                # full-width slots of the last group, then the ragged slot
                nf = last_slots - 1 if w_last < CHUNK else last_slots
                for m in range(nf):
                    dengs[di % len(dengs)].dma_start(
                        out=out[i, :, span0 + m * CHUNK:span0 + (m + 1) * CHUNK],
                        in_=prob2[
                            H * m:H * (m + 1),
                            nfullg * CHUNK:(nfullg + 1) * CHUNK,
                        ],
                    )
                    di += 1
                if w_last < CHUNK:
                    dengs[di % len(dengs)].dma_start(
                        out=out[i, :, span0 + nf * CHUNK:n],
                        in_=prob2[
                            H * nf:H * (nf + 1),
                            nfullg * CHUNK:nfullg * CHUNK + w_last,
                        ],
                    )

    nc.compile()
    return nc


_NC_CACHE = {}


def _get_nc():
    if "nc" not in _NC_CACHE:
        _NC_CACHE["nc"] = build_kernel()
    return _NC_CACHE["nc"]


def kernel(query, key, W, b):
    from concourse.bass_utils import run_bass_kernel_spmd

    query = np.ascontiguousarray(np.asarray(query, np.float32).reshape(B, D))
    key = np.ascontiguousarray(np.asarray(key, np.float32))
    W = np.ascontiguousarray(np.asarray(W, np.float32))
    b = np.ascontiguousarray(np.asarray(b, np.float32))

    nc = _get_nc()
    in_maps = []
    for c in range(NCORES):
        s = slice(BPC * c, BPC * (c + 1))
        in_maps.append(
            {
                "q": query[s],
                "k": key[s],
                "w": W,
                "b": b,
            }
        )
    res = run_bass_kernel_spmd(nc, in_maps, list(range(NCORES))).results
    return np.concatenate([res[c]["out"] for c in range(NCORES)], axis=0)
